# revision 86
# baseline (speedup 1.0000x reference)
"""Trainium2 Bass kernel for nn_MultiHeadAttention_10960756539999.

MHA: inp [2, 2048, 768], 12 heads, head_dim 64, Wqkv [768, 2304] (per-head
192-col slabs [Q|K|V]), Wproj [768, 768].

Sharding: 24 (batch, head) pairs -> 3 heads per core; cores 0-3 take batch 0,
cores 4-7 take batch 1. Host sums the 4 per-batch partials and adds bproj.

All inputs are converted to bf16 on the host (halves DMA traffic; bf16
matmuls run at full PE rate at any moving-free size). Per core:
  - QKV^T for its 3 heads from x^T (PSUM accumulate over the 768-dim).
  - scores^T per (head, query-half): sc_j [128 keys, 1024 q] in PSUM,
    exp on ACT (the hard bottleneck: ~100us of pure exp per core) into
    bf16 SBUF tiles.
  - attV in NATURAL orientation: acc[q, d+ones] += ex_j^T-slices @ V_j.
    65-wide moving ops make this half the PE rows of the transposed form,
    and softmax normalization becomes a per-partition tensor_scalar_mul.
  - PE transposes (via identity) restore out^T for the row-sharded
    projection; partial out in bf16 back to host.
V-phase / next-head QK^T / projection matmuls are interleaved into the
attention j-steps as background PE work so PE never idles while ACT
grinds through the exps.
"""

import sys

import numpy as np

try:
    import concourse.bass as bass
except ImportError:  # harness runs from a bare directory
    sys.path.insert(0, "/opt/trn_rl_repo")
    import concourse.bass as bass

import ml_dtypes

import concourse.tile as tile
from concourse import bacc, mybir
from concourse.bass_utils import run_bass_kernel_spmd

F32 = mybir.dt.float32
BF16 = mybir.dt.bfloat16
AF = mybir.ActivationFunctionType

NH = 12          # total heads
D = 64           # head dim
S = 2048         # sequence length
NI = 768         # model dim
NB = 2           # batch
NCORES = 8
HPC = 3          # heads per core
CPB = NCORES // NB   # cores per batch
KC = NI // 128   # contraction chunks for the 768 dim
NT = S // 128    # 128-row tiles along tokens/keys
HB = S // 2      # query half
SCALE = float(1.0 / np.sqrt(NI / NH))  # 1/8

# filled by kernel() for test.py to report
last_results = None

_cache = {}


def _build_nc(has_bias: bool):
    nc = bacc.Bacc("TRN2", target_bir_lowering=False, debug=False,
                   num_devices=NCORES)

    # xqkv packs [xT chunk | wqk slab | wv slab] per 128-row contraction
    # chunk: one DMA per chunk delivers everything that chunk's matmuls need
    CW = S + HPC * 128 + HPC * D  # 2624
    xqkv_d = nc.dram_tensor("xqkv", [KC, 128, CW], BF16, kind="ExternalInput")
    wp_d = nc.dram_tensor("wp", [HPC * D, NI], BF16, kind="ExternalInput")
    id_d = nc.dram_tensor("ident", [128, 128], BF16, kind="ExternalInput")
    if has_bias:
        # col h: rows 0:64 = bq_h, rows 64:128 = bk_h; bv packed per-head
        bqk_d = nc.dram_tensor("bqk", [128, HPC], F32, kind="ExternalInput")
        bv_d = nc.dram_tensor("bv", [HPC * D], F32, kind="ExternalInput")
    out_d = nc.dram_tensor("out", [S, NI], BF16, kind="ExternalOutput")

    with tile.TileContext(nc) as tc:
        with (
            tc.tile_pool(name="const", bufs=1) as constp,
            tc.tile_pool(name="expp", bufs=5) as expp,
            tc.tile_pool(name="osbp", bufs=14) as osbp,
            tc.tile_pool(name="rwork", bufs=4) as rwork,
            tc.tile_pool(name="ostagep", bufs=4) as ostagep,
            # PSUM: tag "S" = [128,1024] f32 (2 banks) x2, tag "C" =
            # [128,4,65] f32 (1 bank) x2, tag "W" = [128,512] f32 (1 bank)
            # x2 -> 8 banks total.
            tc.tile_pool(name="psS", bufs=2, space="PSUM") as psS,
            tc.tile_pool(name="psC", bufs=2, space="PSUM") as psC,
            tc.tile_pool(name="psW", bufs=2, space="PSUM") as psW,
        ):
            # ---- input loads: wqk_c just before xT_c so the QK^T c-step
            # can start the moment chunk c lands ----
            # packed layout per chunk: [wqk | xT | wv]; the last chunk is
            # split so [wqk | tokens 0:1024] — all that the first scores
            # tile's QK^T needs — lands one DMA piece earlier
            WQ = HPC * 128
            big = constp.tile([128, KC, CW], BF16, tag="xqkv")
            for c in range(KC):
                if c == KC - 1:
                    nc.sync.dma_start(out=big[:, c, 0:WQ + HB],
                                      in_=xqkv_d[c][:, 0:WQ + HB])
                    nc.sync.dma_start(out=big[:, c, WQ + HB:CW],
                                      in_=xqkv_d[c][:, WQ + HB:CW])
                else:
                    nc.sync.dma_start(out=big[:, c, :], in_=xqkv_d[c])
            ident = constp.tile([128, 128], BF16, tag="ident")
            nc.sync.dma_start(out=ident, in_=id_d[:])
            def xTs(c, a, b):
                return big[:, c, WQ + a:WQ + b]

            def wqks(c, h):
                return big[:, c, h * 128:(h + 1) * 128]

            def wvs(c):
                return big[:, c, WQ + S:CW]

            wp01 = constp.tile([128, NI], BF16, tag="wp01")
            wp2 = constp.tile([D, NI], BF16, tag="wp2")
            nc.sync.dma_start(out=wp01, in_=wp_d[0:128, :])
            nc.sync.dma_start(out=wp2, in_=wp_d[128:192, :])

            if has_bias:
                bqk = constp.tile([128, HPC], F32, tag="bqk")
                nc.sync.dma_start(out=bqk, in_=bqk_d[:])
                bvb = constp.tile([128, HPC * D], F32, tag="bvb")
                bv_ap = bv_d[:]
                bv_bcast = bass.AP(
                    tensor=bv_ap.tensor, offset=bv_ap.offset,
                    ap=[[0, 128]] + [list(p) for p in bv_ap.ap])
                nc.sync.dma_start(out=bvb, in_=bv_bcast)

            # V in natural layout, per (key-chunk, head): [128, 65] slabs
            # of [V | ones]; ones column feeds the softmax denominators.
            vall = constp.tile([128, NT, HPC, 65], BF16, tag="vall")
            nc.vector.memset(vall, 1.0)

            qq = [constp.tile([D, S], BF16, tag=f"qq{h}", name=f"qq{h}")
                  for h in range(HPC)]
            kk = [constp.tile([D, S], BF16, tag=f"kk{h}", name=f"kk{h}")
                  for h in range(HPC)]
            oT01 = constp.tile([128, S], BF16, tag="oT01")
            oT2 = constp.tile([D, S], BF16, tag="oT2")

            def qk_copies(h, qt, t0, w, k_on_act=False):
                # GPSIMD cannot read PSUM, so copies land on DVE (the K
                # copy can ride on ACT while it still has pre-exp idle)
                dstq = qq[h][:, t0:t0 + w]
                dstk = kk[h][:, t0:t0 + w]
                if has_bias:
                    nc.vector.tensor_scalar_add(
                        dstq, qt[0:D, :], bqk[0:D, h:h + 1])
                    nc.vector.tensor_scalar_add(
                        dstk, qt[D:128, :], bqk[D:128, h:h + 1])
                else:
                    nc.vector.tensor_copy(dstq, qt[0:D, :])
                    if k_on_act:
                        nc.scalar.copy(dstk, qt[D:128, :])
                    else:
                        nc.vector.tensor_copy(dstk, qt[D:128, :])

            # ---- background PE work units (emitted inside attention
            # j-steps, in program order) ----
            def v_unit(j):
                def emit():
                    pv = psW.tile([128, 512], F32, tag="W", name=f"pv{j}")
                    for c in range(KC):
                        nc.tensor.matmul(
                            pv[:, 0:HPC * D],
                            xTs(c, j * 128, (j + 1) * 128), wvs(c),
                            start=(c == 0), stop=(c == KC - 1))
                    dst = vall[:, j, :, 0:64]
                    src = pv[:, 0:HPC * D].rearrange("p (h d) -> p h d", d=D)
                    if has_bias:
                        bsrc = bvb.rearrange("p (h d) -> p h d", d=D)
                        nc.vector.tensor_add(dst, src, bsrc)
                    else:
                        nc.vector.tensor_copy(dst, src)
                return emit

            def qkt_units(h, rs=(0, 1, 2, 3)):
                # passes of 512 tokens through W slots; one matmul per
                # (pass, chunk) unit so the bucket can pace them finely
                units = []
                state = {}
                for r in rs:
                    for c in range(KC):
                        def mm(h=h, r=r, c=c):
                            if c == 0:
                                state[r] = psW.tile([128, 512], F32, tag="W",
                                                    name=f"qkt{h}_{r}")
                            nc.tensor.matmul(
                                state[r], wqks(c, h),
                                xTs(c, r * 512, (r + 1) * 512),
                                start=(c == 0), stop=(c == KC - 1))
                        units.append((512, mm))

                    def cp(h=h, r=r):
                        qk_copies(h, state[r], r * 512, 512)
                    units.append((150, cp))
                return units

            def proj_units(t, tail=False):
                st = {}

                def fa(t=t):
                    tsl = slice(t * 128, (t + 1) * 128)
                    pa = psW.tile([128, 512], F32, tag="W", name=f"pa{t}")
                    nc.tensor.matmul(pa, oT01[:, tsl], wp01[:, 0:512],
                                     start=True, stop=False)
                    nc.tensor.matmul(pa, oT2[:, tsl], wp2[:, 0:512],
                                     start=False, stop=True)
                    st["pa"] = pa

                def fb(t=t, tail=tail):
                    tsl = slice(t * 128, (t + 1) * 128)
                    pb = psW.tile([128, 512], F32, tag="W", name=f"pb{t}")
                    nc.tensor.matmul(pb[:, 0:256], oT01[:, tsl],
                                     wp01[:, 512:NI], start=True, stop=False)
                    nc.tensor.matmul(pb[:, 0:256], oT2[:, tsl],
                                     wp2[:, 512:NI], start=False, stop=True)
                    ost = ostagep.tile([128, NI], BF16, tag="ost")
                    if tail:
                        # ACT is out of exps by the tail: split the stage
                        # copies across ACT/DVE so the drain pipelines
                        nc.scalar.copy(ost[:, 0:512], st["pa"])
                    else:
                        nc.vector.tensor_copy(ost[:, 0:512], st["pa"])
                    nc.vector.tensor_copy(ost[:, 512:NI], pb[:, 0:256])
                    nc.sync.dma_start(
                        out=out_d[:].rearrange("(t p) o -> t p o", p=128)[t],
                        in_=ost)
                return [(1024, fa), (700, fb)]

            # background units are (pe_rows, fn); a per-j-step token bucket
            # paces them so a step's PE load stays under the ACT exp period
            bg = []          # deadline background units (QK^T, proj)
            tp_pending = []  # deferred transpose/oT-copy units (no deadline)
            bucket = [0.0]
            # PE rows of background budget per j-step: the per-step slack
            # is ~950 rows on paper, but sem latencies / p-state dips eat
            # ~25% — overfeeding makes the pass's own scores land late
            bstep = [950.0]
            BCAP = 2400.0

            def pop_bg(allow_tp=True):
                bucket[0] = min(bucket[0] + bstep[0], BCAP)
                while True:
                    q = bg if bg else (tp_pending if allow_tp else None)
                    if not q or q[0][0] > bucket[0]:
                        break
                    w, fn = q.pop(0)
                    fn()
                    bucket[0] -= w

            # ---- attention passes with a cross-pass scores pipeline:
            # the 2-deep sc queue is global, and each pass's last two
            # j-steps enqueue the NEXT pass's sc(0)/sc(1) so ACT rolls
            # straight through the normalize/transpose chain at pass
            # boundaries ----
            sc_queue = []

            def sc_mms(h, qh, j):
                sc = psS.tile([128, HB], F32, tag="S", name="sc")
                klhs = kk[h][:, j * 128:(j + 1) * 128]
                for n in range(2):
                    sl = slice(qh * HB + n * 512, qh * HB + (n + 1) * 512)
                    nc.tensor.matmul(
                        sc[:, n * 512:(n + 1) * 512], klhs, qq[h][:, sl])
                return sc

            def att_pass(h, qh, nxt):
                ca = psC.tile([128, 512], F32, tag="C", name=f"ac{h}{qh}a")
                cb = psC.tile([128, 512], F32, tag="C", name=f"ac{h}{qh}b")
                acc_a = ca[:, 0:260].rearrange("p (q c) -> p q c", c=65)
                acc_b = cb[:, 0:260].rearrange("p (q c) -> p q c", c=65)

                def attv_mms(j, ex):
                    vrhs = vall[:, j, h, :]
                    for qc in range(8):
                        acc = acc_a if qc < 4 else acc_b
                        qi = qc % 4
                        nc.tensor.matmul(
                            acc[:, qi, :], ex[:, qc * 128:(qc + 1) * 128],
                            vrhs,
                            start=(j == 0 and qi == 0), stop=(j == NT - 1),
                            skip_group_check=True)

                for j in range(NT):
                    ex = expp.tile([128, HB], BF16, tag="exp")
                    nc.scalar.activation(ex, sc_queue.pop(0), AF.Exp,
                                         scale=SCALE)
                    if j + 2 < NT:
                        sc_queue.append(sc_mms(h, qh, j + 2))
                    elif nxt is not None:
                        sc_queue.append(sc_mms(nxt[0], nxt[1], j + 2 - NT))
                    if h == 0 and qh == 0 and j + 3 < NT:
                        # V tile j+3 is due at step j+3: mandatory, un-paced
                        # (v0-v2 ran eagerly, so the pass's last steps are
                        # unloaded right where the boundary chain starts)
                        v_unit(j + 3)()
                    pop_bg()
                    attv_mms(j, ex)

                # normalize: all the DVE muls first (frees the acc banks
                # promptly, no PE<->DVE ping-pong), then the transposes +
                # oT copies run as deferred units inside the NEXT pass's
                # j-steps; on the last pass they run inline, each token
                # tile draining straight into its projection
                tp_units = []
                for g, ct in ((0, ca), (1, cb)):
                    # one staging copy frees the acc bank immediately (the
                    # next pass's attV start waits on it); the muls then
                    # read SBUF (58-cycle access) instead of PSUM (120)
                    ust = rwork.tile([128, 260], F32, tag="ust", bufs=4)
                    nc.vector.tensor_copy(ust, ct[:, 0:260])
                    acc = ust.rearrange("p (q c) -> p q c", c=65)
                    rcp = rwork.tile([128, 4], F32, tag="rcp")
                    nc.vector.reciprocal(rcp, acc[:, :, 64])
                    for qi in range(4):
                        qc = g * 4 + qi
                        t = qh * 8 + qc  # global token tile
                        osb = osbp.tile([128, D], BF16, tag="osb")
                        nc.vector.tensor_scalar_mul(
                            osb, acc[:, qi, 0:D], rcp[:, qi:qi + 1])

                        def tp_u(t=t, osb=osb, h=h, last=(nxt is None)):
                            tp = psW.tile([128, 512], F32, tag="W", name="tp")
                            tpv = tp[0:D, 0:D].bitcast(BF16)  # [64,128] bf16
                            nc.tensor.transpose(tpv, osb, ident)
                            tsl = slice(t * 128, (t + 1) * 128)
                            if h < 2:
                                dst = oT01[h * D:(h + 1) * D, tsl]
                            else:
                                dst = oT2[:, tsl]
                            nc.vector.tensor_copy(dst, tpv)
                            if last:
                                for _, fn in proj_units(t, tail=True):
                                    fn()
                        tp_units.append((250, tp_u))
                if nxt is None:
                    for _, u in tp_units:
                        u()
                    return []
                return tp_units

            # ---- startup: QK^T h0 (both halves) and the first two V
            # tiles run c-outer so PE consumes each xqkv chunk the moment
            # it lands ----
            qk_a = psS.tile([128, HB], F32, tag="S", name="qk0a")
            qk_b = psS.tile([128, HB], F32, tag="S", name="qk0b")
            for c in range(KC):
                for half, qt in ((0, qk_a), (1, qk_b)):
                    for n in range(2):
                        a = half * HB + n * 512
                        nc.tensor.matmul(
                            qt[:, n * 512:(n + 1) * 512],
                            wqks(c, 0), xTs(c, a, a + 512),
                            start=(c == 0), stop=(c == KC - 1))
            # sc(0) needs only qq half 0 + kk chunk 0; the half-0 K copy
            # rides on the still-idle ACT
            qk_copies(0, qk_a, 0, HB, k_on_act=True)
            sc_queue.append(sc_mms(0, 0, 0))
            # K half-1 on ACT too: sc(1) waits qk_b's slot release (both
            # copies), and DVE alone would serialize them behind qa
            qk_copies(0, qk_b, HB, HB, k_on_act=True)
            sc_queue.append(sc_mms(0, 0, 1))
            v_unit(0)()
            v_unit(1)()
            v_unit(2)()

            # ---- pass schedule: h-outer; V mandatory during (h0,qh0);
            # QKT h+1 during (h,qh1); deferred transposes fill the slack;
            # (h2,qh1) interleaves (2,0)'s transposes with proj tiles 0-7;
            # tail proj 8-15 drains inline from the last normalize ----
            passes = [(h, qh) for h in range(HPC) for qh in range(2)]
            bg_feed = {
                (0, 1): qkt_units(1),
                (1, 0): qkt_units(2),
            }
            prev_tp = []
            for idx, (h, qh) in enumerate(passes):
                nxt = passes[idx + 1] if idx + 1 < len(passes) else None
                bstep[0] = 950.0
                if (h, qh) == (2, 1):
                    # proj tile t needs (2,0)'s transpose t: interleave
                    for i, tpu in enumerate(prev_tp):
                        bg.append(tpu)
                        bg.extend(proj_units(i))
                else:
                    tp_pending.extend(prev_tp)
                    bg.extend(bg_feed.get((h, qh), []))
                prev_tp = att_pass(h, qh, nxt)
            while bg or tp_pending:
                bucket[0] = BCAP
                pop_bg()

    nc.compile()
    return nc


def _get_nc(has_bias: bool):
    if has_bias not in _cache:
        _cache[has_bias] = _build_nc(has_bias)
    return _cache[has_bias]


def kernel(inp, Wqkv, bqkv, Wproj, bproj):
    global last_results
    inp = np.ascontiguousarray(np.asarray(inp, dtype=np.float32))
    Wqkv = np.asarray(Wqkv, dtype=np.float32)
    bqkv = np.asarray(bqkv, dtype=np.float32)
    Wproj = np.asarray(Wproj, dtype=np.float32)
    bproj = np.asarray(bproj, dtype=np.float32)
    assert inp.shape == (NB, S, NI), inp.shape

    has_bias = bool(np.any(bqkv))
    nc = _get_nc(has_bias)

    bf = ml_dtypes.bfloat16
    CW = S + HPC * 128 + HPC * D
    xTs = [np.ascontiguousarray(inp[b].T).astype(bf) for b in range(NB)]
    ident = np.eye(128, dtype=bf)

    in_maps = []
    for core in range(NCORES):
        b = core // CPB
        heads = [(core % CPB) * HPC + i for i in range(HPC)]
        wqk = np.empty((NI, HPC * 128), np.float32)
        wv = np.empty((NI, HPC * D), np.float32)
        wp = np.empty((HPC * D, NI), np.float32)
        for i, h in enumerate(heads):
            base = h * 3 * D
            wqk[:, i * 128:i * 128 + D] = Wqkv[:, base:base + D]
            wqk[:, i * 128 + D:(i + 1) * 128] = Wqkv[:, base + D:base + 2 * D]
            wv[:, i * D:(i + 1) * D] = Wqkv[:, base + 2 * D:base + 3 * D]
            wp[i * D:(i + 1) * D, :] = Wproj[h * D:(h + 1) * D, :]
        # pack [wqk | xT | wv] per 128-row contraction chunk
        xqkv = np.empty((KC, 128, CW), bf)
        xqkv[:, :, 0:HPC * 128] = \
            wqk.astype(bf).reshape(KC, 128, HPC * 128)
        xqkv[:, :, HPC * 128:HPC * 128 + S] = xTs[b].reshape(KC, 128, S)
        xqkv[:, :, HPC * 128 + S:CW] = \
            wv.astype(bf).reshape(KC, 128, HPC * D)
        m = {"xqkv": xqkv, "wp": wp.astype(bf), "ident": ident}
        if has_bias:
            bqk = np.empty((128, HPC), np.float32)
            bv = np.empty((HPC * D,), np.float32)
            for i, h in enumerate(heads):
                base = h * 3 * D
                bqk[0:D, i] = bqkv[base:base + D]
                bqk[D:128, i] = bqkv[base + D:base + 2 * D]
                bv[i * D:(i + 1) * D] = bqkv[base + 2 * D:base + 3 * D]
            m["bqk"] = bqk
            m["bv"] = bv
        in_maps.append(m)

    res = run_bass_kernel_spmd(nc, in_maps, core_ids=list(range(NCORES)))
    last_results = res

    out = np.zeros((NB, S, NI), np.float32)
    for core in range(NCORES):
        out[core // CPB] += res.results[core]["out"].astype(np.float32)
    out += bproj
    return out
